# revision 22
# baseline (speedup 1.0000x reference)
"""EnhancedSTGCN Trainium2 kernel.

Data-parallel over batch N=128 across 8 NeuronCores (16 samples/core).
Per-core Bass/Tile kernel computes the full ST-GCN pipeline:
  data_bn -> 3x [GraphConv -> BN+ReLU -> tconv(9x1) -> BN + residual -> ReLU -> EMA]
  -> global mean pool -> FC.

Activation layout on-chip: [channels(partitions), t*18+v (free)] per sample.
The GraphConv V-contraction runs on the tensor engine via a transposed-chunk
trick: matmul(lhsT=x_chunk[ci,126]) puts (7t,18v) on PSUM partitions, then a
second matmul against block-diagonal I7 (x) A^T applies the adjacency and
transposes back to [co, 126] in one instruction.

Matmul operands are bf16 (fp32 PSUM accumulation); elementwise chain
(BN/residual/ReLU/EMA-scan) runs in fp32. Set MM_MODE="f32" for full fp32.

All constant weights ship as ONE packed DRAM tensor -> one DMA -> one
semaphore (walrus limits sync-waits per instruction; scattered weight DMAs
across HWDGE queues blow that limit).
"""

import sys

import numpy as np

sys.path.insert(0, "/opt/trn_rl_repo")

V = 18
T0 = 256
NS = 16  # samples per core
N_CORES = 8
ALPHA = 0.85
EPS = 1e-5
MM_MODE = "bf16"  # "bf16" | "f32"

# (ci, co, T_in, stride) per block
BLOCKS = [(2, 64, 256, 1), (64, 128, 256, 2), (128, 256, 128, 2)]

_NC_CACHE = {}


def _mm_np_dtype():
    if MM_MODE == "bf16":
        import ml_dtypes
        return ml_dtypes.bfloat16
    return np.float32


def _wlayout():
    """Packed weight layout: list of (name, rows, f32cols, kind) + offsets.

    kind: "mm" (matmul dtype: bf16 pairs packed in f32 container) | "f32".
    """
    per = 2 if MM_MODE == "bf16" else 4  # bytes/elem
    def mmcols(width):  # f32 columns for `width` mm-dtype elements
        return width * per // 4

    entries = [("aexp", 128, mmcols(128), "mm")]
    for l, (ci, co, T, stride) in enumerate(BLOCKS, 1):
        cop = min(co, 128)
        mhc = (co + 127) // 128
        khc = mhc
        entries.append((f"gwT{l}", ci, mmcols(co), "mm"))
        entries.append((f"twT{l}", cop, mmcols(9 * khc * co), "mm"))
        entries.append((f"rwT{l}", ci, mmcols(co), "mm"))
        for nm in ("s1", "b1", "b2c", "b2c015"):
            entries.append((f"{nm}_{l}", cop, mhc, "f32"))
    entries.append(("fcwT", 128, 20, "f32"))
    entries.append(("fcb", 1, 10, "f32"))
    off = 0
    layout = {}
    for name, rows, cols, kind in entries:
        layout[name] = (rows, off, cols, kind)
        off += cols
    return layout, off


def _build_nc(ns=NS):
    import concourse.bass as bass
    import concourse.tile as tile
    from concourse import bacc, mybir
    from contextlib import ExitStack

    F32 = mybir.dt.float32
    MDT = mybir.dt.bfloat16 if MM_MODE == "bf16" else F32
    AF = mybir.ActivationFunctionType
    OP = mybir.AluOpType

    layout, wtot = _wlayout()

    # Bacc (not raw Bass): its compile() runs generate_event_semaphores(),
    # which splits sync waits to <=1 per instruction (TRN2 HW constraint).
    nc = bacc.Bacc("TRN2", target_bir_lowering=False, debug=False)

    # ---- DRAM I/O ----
    # x arrives with data_bn pre-applied (host-side affine fold), in matmul dtype
    x_d = nc.dram_tensor("x", [ns, 2, T0 * V], MDT, kind="ExternalInput")
    wpack_d = nc.dram_tensor("wpack", [128, wtot], F32, kind="ExternalInput")
    out_d = nc.dram_tensor("out", [ns, 10], F32, kind="ExternalOutput")

    with ExitStack() as ctx:
        tc = ctx.enter_context(tile.TileContext(nc))
        wp = ctx.enter_context(tc.tile_pool(name="wp", bufs=1))

        wtile = wp.tile([128, wtot], F32)
        nc.sync.dma_start(wtile[:], wpack_d[:])

        def wview(name):
            rows, off, cols, kind = layout[name]
            v = wtile[0:rows, off: off + cols]
            if kind == "mm" and MM_MODE == "bf16":
                v = v.bitcast(MDT)
            return v

        aexp = wview("aexp")
        gw_s = {l: wview(f"gwT{l}") for l in (1, 2, 3)}
        tw_s = {l: wview(f"twT{l}") for l in (1, 2, 3)}
        rw_s = {l: wview(f"rwT{l}") for l in (1, 2, 3)}
        s1_s = {l: wview(f"s1_{l}") for l in (1, 2, 3)}
        b1_s = {l: wview(f"b1_{l}") for l in (1, 2, 3)}
        b2c_s = {l: wview(f"b2c_{l}") for l in (1, 2, 3)}
        b2c015_s = {l: wview(f"b2c015_{l}") for l in (1, 2, 3)}
        fcw_s = wview("fcwT")
        fcb_s = wview("fcb")

        alpha_t = wp.tile([128, T0], F32)
        nc.vector.memset(alpha_t[:], ALPHA)
        ones_t = wp.tile([1, ns], F32)
        nc.vector.memset(ones_t[:], 1.0)
        pooled = wp.tile([128, 2 * ns], F32)

        def chunk_list(total, step):
            full, rem = divmod(total, step)
            out = [(i * step, step) for i in range(full)]
            if rem:
                out.append((full * step, rem))
            return out

        with (
            tc.tile_pool(name="xp", bufs=2) as xp,
            tc.tile_pool(name="actp", bufs=1) as actp,
            tc.tile_pool(name="outp", bufs=2) as outp,
            tc.tile_pool(name="smp", bufs=3) as smp,
            tc.tile_pool(name="aps", bufs=2, space="PSUM") as aps,
            tc.tile_pool(name="bps", bufs=2, space="PSUM") as bps,
        ):
            def block(n, l, xin, ci, co, T, stride):
                Tp = T // stride
                mhc = (co + 127) // 128
                khc = mhc
                cop = min(co, 128)
                pdw = (T + 8) * V

                pd = actp.tile([cop, mhc * pdw], MDT, tag="pd", name=f"pd{l}_{n}")
                for mh in range(mhc):
                    nc.gpsimd.memset(pd[:, mh * pdw: mh * pdw + 4 * V], 0.0)
                    nc.gpsimd.memset(pd[:, mh * pdw + (T + 4) * V: (mh + 1) * pdw], 0.0)

                # ---- A-phase: graph conv (channel mix + adjacency) ----
                for (t0, tcn) in chunk_list(T, 7):
                    P = tcn * V
                    ps1 = aps.tile([126, co], F32, tag="ps1", name=f"ps1_{l}_{n}_{t0}")
                    nc.tensor.matmul(ps1[0:P, :], xin[:, t0 * V: t0 * V + P],
                                     gw_s[l], start=True, stop=True)
                    y1 = smp.tile([126, co], MDT, tag="y1", name=f"y1_{l}_{n}_{t0}")
                    nc.vector.tensor_copy(y1[0:P, :], ps1[0:P, :])
                    for mh in range(mhc):
                        ps3 = aps.tile([cop, 126], F32, tag="ps3",
                                       name=f"ps3_{l}_{n}_{t0}_{mh}")
                        nc.tensor.matmul(ps3[:, 0:P],
                                         y1[0:P, mh * 128: mh * 128 + cop],
                                         aexp[0:P, 0:P], start=True, stop=True)
                        nc.scalar.activation(
                            pd[:, mh * pdw + (4 + t0) * V: mh * pdw + (4 + t0 + tcn) * V],
                            ps3[:, 0:P], AF.Relu,
                            bias=b1_s[l][:, mh: mh + 1], scale=s1_s[l][:, mh: mh + 1])

                # ---- B-phase: temporal conv + residual, one PSUM accumulation
                # group per chunk (s2 folded into tw host-side); d1 = relu(0.15u+b)
                # comes straight off PSUM via one ACT op.
                us = actp.tile([cop, mhc * Tp * V], F32, tag="us", name=f"us{l}_{n}")
                inits = {}
                xin3 = xin.rearrange("p (t v) -> p t v", v=V)
                for (t0, tcn) in chunk_list(Tp, 28):
                    NC = tcn * V
                    for mh in range(mhc):
                        pstc = bps.tile([cop, 504], F32, tag="pstc", bufs=3,
                                        name=f"pstc{l}_{n}_{t0}_{mh}")
                        rr = xin3[:, t0 * stride: (t0 + tcn - 1) * stride + 1: stride, :]
                        nc.tensor.matmul(pstc[:, 0:NC],
                                         rw_s[l][:, mh * 128: mh * 128 + cop],
                                         rr, start=True, stop=False)
                        nmm = 9 * khc
                        i = 0
                        for k in range(9):
                            for kh in range(khc):
                                pdsec = pd[:, kh * pdw: (kh + 1) * pdw].rearrange(
                                    "p (t v) -> p t v", v=V)
                                rhs = pdsec[:, stride * t0 + k:
                                            stride * t0 + k + (tcn - 1) * stride + 1: stride, :]
                                woff = (k * khc + kh) * co + mh * 128
                                nc.tensor.matmul(pstc[:, 0:NC],
                                                 tw_s[l][:, woff: woff + cop],
                                                 rhs,
                                                 start=False, stop=(i == nmm - 1))
                                i += 1
                        if t0 == 0:
                            init = smp.tile([128, V], F32, tag="init",
                                            name=f"init{l}_{n}_{mh}")
                            nc.scalar.activation(init[0:cop, :], pstc[:, 0:V],
                                                 AF.Relu, bias=b2c_s[l][:, mh: mh + 1])
                            inits[mh] = init
                        nc.scalar.activation(
                            us[:, mh * Tp * V + t0 * V: mh * Tp * V + t0 * V + NC],
                            pstc[:, 0:NC], AF.Relu,
                            bias=b2c015_s[l][:, mh: mh + 1], scale=1.0 - ALPHA)

                # ---- C-phase: EMA smooth (scan over t) ----
                ot = outp.tile([cop, mhc * Tp * V], MDT, tag="out", name=f"out{l}_{n}")
                for mh in range(mhc):
                    init = inits[mh]
                    osec = ot[:, mh * Tp * V: (mh + 1) * Tp * V]
                    nc.vector.tensor_copy(osec[:, 0:V], init[0:cop, :])
                    o3 = osec.rearrange("p (t v) -> p t v", v=V)
                    d3 = us[:, mh * Tp * V: (mh + 1) * Tp * V].rearrange(
                        "p (t v) -> p t v", v=V)
                    for v in range(V):
                        nc.vector.tensor_tensor_scan(
                            o3[:, 1:Tp, v], alpha_t[0:cop, 0:Tp - 1], d3[:, 1:Tp, v],
                            init[0:cop, v: v + 1], OP.mult, OP.add)
                return ot

            for n in range(ns):
                x_sb = xp.tile([2, T0 * V], MDT, tag="x", name=f"x_{n}")
                nc.sync.dma_start(x_sb[:], x_d[n])
                h = x_sb
                for l, (ci, co, T, stride) in enumerate(BLOCKS, 1):
                    h = block(n, l, h, ci, co, T, stride)
                # global mean pool (sum; 1/(64*18) folded into fc weights)
                for mh in range(2):
                    nc.vector.tensor_reduce(
                        pooled[:, mh * ns + n: mh * ns + n + 1],
                        h[:, mh * 64 * V: (mh + 1) * 64 * V],
                        axis=mybir.AxisListType.X, op=OP.add)

        # ---- FC head ----
        with tc.tile_pool(name="fcps", bufs=1, space="PSUM") as fcps, \
             tc.tile_pool(name="fcout", bufs=1) as fcout:
            ps = fcps.tile([ns, 10], F32)
            nc.tensor.matmul(ps[:], pooled[:, 0:ns], fcw_s[:, 0:10],
                             start=True, stop=False)
            nc.tensor.matmul(ps[:], pooled[:, ns: 2 * ns], fcw_s[:, 10:20],
                             start=False, stop=False)
            nc.tensor.matmul(ps[:], ones_t[:], fcb_s[:], start=False, stop=True)
            osb = fcout.tile([ns, 10], F32)
            nc.scalar.copy(osb[:], ps[:])
            nc.sync.dma_start(out_d[:], osb[:])

    nc.compile()
    return nc


def _host_inputs(inputs, ns=NS):
    """Build the single packed weight tensor (replicated across cores)."""
    f32 = np.float32
    mdt = _mm_np_dtype()
    layout, wtot = _wlayout()
    wpack = np.zeros((128, wtot), f32)

    def put(name, arr):
        rows, off, cols, kind = layout[name]
        if kind == "mm":
            arr = np.ascontiguousarray(arr.astype(mdt))
            if MM_MODE == "bf16":
                assert arr.shape[-1] % 2 == 0
                wpack.view(np.uint32)[0:rows, off: off + cols] = arr.view(np.uint32)
                return
        arr = np.ascontiguousarray(arr.astype(f32))
        wpack[0:rows, off: off + cols] = arr

    A = np.asarray(inputs["A"], f32)
    aexp = np.zeros((128, 128), f32)
    for t in range(7):
        aexp[t * V:(t + 1) * V, t * V:(t + 1) * V] = A.T
    put("aexp", aexp)
    for l, (ci, co, T, stride) in enumerate(BLOCKS, 1):
        cop = min(co, 128)
        mhc = (co + 127) // 128
        khc = mhc
        gw = np.asarray(inputs[f"l{l}_gw"], f32)
        tw = np.asarray(inputs[f"l{l}_tw"], f32)
        rw = np.asarray(inputs[f"l{l}_rw"], f32)[:, :, 0, 0]
        g1 = np.asarray(inputs[f"l{l}_bn1g"], f32)
        bb1 = np.asarray(inputs[f"l{l}_bn1b"], f32)
        gb = np.asarray(inputs[f"l{l}_gb"], f32)
        g2 = np.asarray(inputs[f"l{l}_bn2g"], f32)
        bb2 = np.asarray(inputs[f"l{l}_bn2b"], f32)
        tb = np.asarray(inputs[f"l{l}_tb"], f32)
        rb = np.asarray(inputs[f"l{l}_rb"], f32)
        s1 = g1 / np.sqrt(f32(1.0) + f32(EPS))
        b1v = s1 * gb + bb1
        s2 = g2 / np.sqrt(f32(1.0) + f32(EPS))
        b2c = s2 * tb + bb2 + rb
        b2c015 = f32(1.0 - ALPHA) * b2c
        put(f"gwT{l}", gw.T)
        tws = tw * s2[:, None, None, None]  # fold bn2 scale into tconv weights
        twp = np.zeros((cop, 9 * khc * co), f32)
        for k in range(9):
            for kh in range(khc):
                blk = tws[:, kh * 128: kh * 128 + cop, k, 0].T  # [cop, co]
                twp[:, (k * khc + kh) * co:(k * khc + kh + 1) * co] = blk
        put(f"twT{l}", twp)
        put(f"rwT{l}", rw.T)
        for nm, vec in (("s1", s1), ("b1", b1v), ("b2c", b2c), ("b2c015", b2c015)):
            put(f"{nm}_{l}", np.ascontiguousarray(vec.reshape(mhc, cop).T))
    fcw = np.asarray(inputs["fc_w"], f32)  # [10, 256]
    fcwT = fcw.T / f32(64 * V)  # fold mean pool
    put("fcwT", np.concatenate([fcwT[0:128, :], fcwT[128:256, :]], axis=1))
    put("fcb", np.asarray(inputs["fc_b"], f32).reshape(1, 10))
    return {"wpack": wpack}


def _host_x(inputs):
    """Apply data_bn (eval-mode affine, host fold) and cast to matmul dtype."""
    f32 = np.float32
    x = np.asarray(inputs["x"], f32)  # (N, 2, 256, 18)
    s = (np.asarray(inputs["dbn_g"], f32)
         / np.sqrt(f32(1.0) + f32(EPS))).reshape(2, V)
    b = np.asarray(inputs["dbn_b"], f32).reshape(2, V)
    xb = x * s[None, :, None, :] + b[None, :, None, :]
    return np.ascontiguousarray(xb.reshape(x.shape[0], 2, T0 * V)).astype(_mm_np_dtype())


def kernel(**inputs) -> np.ndarray:
    from concourse.bass_utils import run_bass_kernel_spmd

    n_total = np.asarray(inputs["x"]).shape[0]
    ns = n_total // N_CORES
    key = ("nc", ns)
    if key not in _NC_CACHE:
        _NC_CACHE[key] = _build_nc(ns)
    nc = _NC_CACHE[key]

    shared = _host_inputs(inputs, ns)
    xb = _host_x(inputs)
    in_maps = []
    for c in range(N_CORES):
        m = dict(shared)
        m["x"] = np.ascontiguousarray(xb[c * ns:(c + 1) * ns])
        in_maps.append(m)

    res = run_bass_kernel_spmd(nc, in_maps, core_ids=list(range(N_CORES)))
    return np.concatenate([res.results[c]["out"] for c in range(N_CORES)], axis=0)
